# revision 1
# baseline (speedup 1.0000x reference)
"""Trainium2 Bass kernel for CrossAttention (LN -> QKV proj -> MHA -> out proj).

Sharding: data-parallel over (batch, query-half): 8 shards for B=4.
Each core gets a [1024, 1024] query-token slice and the full [2048, 768]
context for its batch, and produces a [1024, 1024] output slice.

Per-core dataflow (matmul operands bf16, accumulation fp32 in PSUM):
  - LayerNorm on query/context tokens in natural [tok, C] layout (DVE
    bn_stats / bn_aggr), gamma/beta applied with partition-broadcast rows.
  - Transpose LN'd activations to channel-major [C, tok] via DMA xbar
    transpose (bf16) so channels sit on the contraction (partition) axis.
    Activations/projections are chunked (512 tokens) so projections start
    while later chunks are still normalizing.
  - QT = Wq^T @ XqT, KT = Wk^T @ XcT (channel-major), V natural [tok, C].
  - Attention runs per head-quad: scoresT[k, q] = K_h @ Q_h^T (contraction
    D=64; even/odd heads at partitions 0-63/64-127 land on disjoint PE row
    groups and overlap), exp on ACT straight out of PSUM (scores are
    bounded, so no max subtraction).  attendedT accumulates per head pair
    into one [128, 512] psum via column tiling (head0 rows 0-63, head1
    rows 64-127, concurrent).  Softmax denominators come from ones-vector
    matmuls, 4 heads packed at output partitions 0/32/64/96 of one psum.
  - Normalize with DVE reciprocal + DRAM-bounce partition broadcast, then
    out = attendedT^T @ Wo + bo in natural layout, DMA out.
"""

import numpy as np

import concourse.bass as bass
import concourse.tile as tile
from concourse import mybir
from concourse.bass_utils import run_bass_kernel_spmd

F32 = mybir.dt.float32
BF16 = mybir.dt.bfloat16
AF = mybir.ActivationFunctionType
OP = mybir.AluOpType

B, NQ_FULL, NK, CQ, CK, H, D = 4, 2048, 2048, 1024, 768, 16, 64
NQ = 1024            # per-core query tokens
N_CORES = 8
EPS = 1e-5
SM_SCALE = 1.0 / np.sqrt(D)  # 0.125

KC_Q = CQ // 128     # 8  contraction chunks for CQ
KC_C = CK // 128     # 6  contraction chunks for CK
NQT = NQ // 128      # 8  query token tiles
NKT = NK // 128      # 16 context token tiles
QC = 512             # q processed in chunks of 512 (psum free-dim limit)
NQ2 = NQ // QC       # 2


def _split_excess_waits(nc, max_waits=1):
    """walrus in this container accepts at most one sync wait per
    instruction; Tile's kernel-tail drain carries several.  Hoist excess
    waits onto single-wait NOPs that precede the instruction on the same
    engine (absolute sem waits commute, so this is semantics-preserving)."""
    for fn in nc.m.functions:
        for blk in fn.blocks:
            out = []
            dirty = False
            for inst in list(blk.instructions):
                si = inst.sync_info
                if si is not None and len(si.on_wait) > max_waits:
                    waits = list(si.on_wait)
                    for k, w in enumerate(waits[:-max_waits]):
                        nop = mybir.InstNoOp(
                            name=f"wsplit-{inst.name}-{k}", ins=[], outs=[])
                        nop.engine = inst.engine
                        nop.sync_info = mybir.SyncInfo(on_wait=[w], on_update=[])
                        out.append(nop)
                    inst.sync_info = mybir.SyncInfo(
                        on_wait=waits[-max_waits:], on_update=list(si.on_update))
                    dirty = True
                out.append(inst)
            if dirty:
                blk.instructions = out


def _bcast_ap(handle, n_parts, n_free):
    """DRAM [n_free] vector replicated across n_parts partitions."""
    return bass.AP(tensor=handle.ap().tensor, offset=0,
                   ap=[[0, n_parts], [1, n_free]])


def _emit(tc, t, out, stages=("proj", "attn", "out")):
    from contextlib import ExitStack
    nc = tc.nc

    es = ExitStack()
    persist = es.enter_context(tc.tile_pool(name="persist", bufs=1))

    # chunked persistent tensors (distinct tags => distinct slots)
    # kT split per (channel-chunk, token-chunk): finer deps let attention
    # start while later context chunks are still projecting
    qTc = [persist.tile([128, NQ], BF16, tag=f"qT{oc}", name=f"qT{oc}")
           for oc in range(KC_Q)]
    kTc = [[persist.tile([128, QC], BF16, tag=f"kT{oc}_{t4}",
                         name=f"kT{oc}_{t4}") for t4 in range(NK // QC)]
           for oc in range(KC_Q)]
    # V with a ones column appended per head: the attended-value matmul
    # then also emits the softmax denominator (row 64 of its psum)
    v_g = [persist.tile([128, 4, H, D + 1], BF16, tag=f"v{g}", name=f"v{g}")
           for g in range(NKT // 4)]
    bq_cols = persist.tile([128, KC_Q], F32)
    bk_cols = persist.tile([128, KC_Q], F32)
    bvb = persist.tile([128, CQ], F32)
    eps_t = persist.tile([128, 1], F32)

    nc.vector.memset(eps_t[:, :], EPS)

    # ---------------- phase 1+2: LN, transpose, projections ----------------
    with tc.tile_pool(name="pps", bufs=3, space="PSUM") as pps, \
         tc.tile_pool(name="lnw", bufs=1) as lnw, \
         tc.tile_pool(name="xfp", bufs=3) as xfp, \
         tc.tile_pool(name="stp", bufs=4) as stp, \
         tc.tile_pool(name="bfp", bufs=4) as bfp, \
         tc.tile_pool(name="wfp", bufs=3) as wfp, \
         tc.tile_pool(name="xTp", bufs=2) as xTp, \
         tc.tile_pool(name="wbp", bufs=1) as wbp, \
         tc.tile_pool(name="scrb", bufs=1, space="DRAM") as scrb:

        # gamma/beta as [128, n_chunk] columns; LN fold:
        # LN(x)@W + b = ((x-mu)*rstd) @ (gamma.W) + (b + (beta/gamma)@(gamma.W))
        gq = lnw.tile([128, KC_Q], F32, name="gq")
        gc = lnw.tile([128, KC_C], F32, name="gc")
        btq = lnw.tile([128, KC_Q], F32, name="btq")
        btc = lnw.tile([128, KC_C], F32, name="btc")
        nc.gpsimd.dma_start(out=gq[:, :],
                            in_=t["gamma_q"].ap().rearrange("(j p) -> p j", p=128))
        nc.gpsimd.dma_start(out=gc[:, :],
                            in_=t["gamma_ctx"].ap().rearrange("(j p) -> p j", p=128))
        nc.gpsimd.dma_start(out=btq[:, :],
                            in_=t["beta_q"].ap().rearrange("(j p) -> p j", p=128))
        nc.gpsimd.dma_start(out=btc[:, :],
                            in_=t["beta_ctx"].ap().rearrange("(j p) -> p j", p=128))
        btq8 = lnw.tile([128, KC_Q], BF16, name="btq8")
        btc8 = lnw.tile([128, KC_C], BF16, name="btc8")
        rg = lnw.tile([128, KC_Q], F32, name="rg")
        nc.vector.reciprocal(out=rg[:, :KC_Q], in_=gq[:, :])
        nc.vector.tensor_mul(out=btq8[:, :], in0=btq[:, :], in1=rg[:, :KC_Q])
        nc.vector.reciprocal(out=rg[:, :KC_C], in_=gc[:, :])
        nc.vector.tensor_mul(out=btc8[:, :], in0=btc[:, :], in1=rg[:, :KC_C])

        def ln_tile(x_dram, i, C, n_sub, sub, xT_chunk, col0):
            """LN one [128, C] token tile (gamma/beta folded into the
            weights), write bf16 transpose into xT_chunk[kc]."""
            xf = xfp.tile([128, C], F32, tag="xf", name=f"xf_{i}_{C}")
            nc.scalar.dma_start(out=xf[:, :],
                                in_=x_dram.ap()[i * 128:(i + 1) * 128, :])
            st = stp.tile([128, n_sub, 6], F32, tag="st", name=f"st_{i}_{C}")
            for s in range(n_sub):
                nc.vector.bn_stats(out=st[:, s, :],
                                   in_=xf[:, s * sub:(s + 1) * sub])
            mv = stp.tile([128, 2], F32, tag="mv", name=f"mv_{i}_{C}")
            nc.vector.bn_aggr(out=mv[:, :], in_=st[:, :, :])
            nc.scalar.activation(out=mv[:, 1:2], in_=mv[:, 1:2],
                                 func=AF.Sqrt, bias=eps_t[:, :], scale=1.0)
            nc.vector.reciprocal(out=mv[:, 1:2], in_=mv[:, 1:2])
            xbf = bfp.tile([128, C], BF16, tag="xbf", name=f"xbf_{i}_{C}")
            nc.vector.tensor_scalar(out=xbf[:, :], in0=xf[:, :],
                                    scalar1=mv[:, 0:1], scalar2=mv[:, 1:2],
                                    op0=OP.subtract, op1=OP.mult)
            for j in range(C // 128):
                nc.sync.dma_start(out=xT_chunk[j][:, col0:col0 + 128],
                                  in_=xbf[:, j * 128:(j + 1) * 128],
                                  transpose=True)

        def load_w(dram, n_chunks, tagp, gcol):
            tiles = []
            for kc in range(n_chunks):
                wf = wfp.tile([128, CQ], F32, tag="wf", name=f"wf{tagp}{kc}")
                nc.scalar.dma_start(out=wf[:, :],
                                    in_=dram.ap()[kc * 128:(kc + 1) * 128, :])
                wb = wbp.tile([128, CQ], BF16, tag=f"w{tagp}{kc}",
                              name=f"w{tagp}{kc}")
                nc.gpsimd.tensor_scalar_mul(out=wb[:, :], in0=wf[:, :],
                                            scalar1=gcol[:, kc:kc + 1])
                tiles.append(wb)
            return tiles

        def bias_fold(tiles, bname, n_kc, btcol, tagp, scr_pool):
            """bias' = b + beta~ @ W' -> DRAM row [CQ]."""
            brow = lnw.tile([1, CQ], F32, tag="brow",
                            name=f"brow{tagp}")
            nc.gpsimd.dma_start(out=brow[:, :],
                                in_=bass.AP(tensor=t[bname].ap().tensor,
                                            offset=0, ap=[[0, 1], [1, CQ]]))
            bsum = lnw.tile([1, CQ], F32, tag="bsum",
                            name=f"bsum{tagp}")
            for half in range(2):
                ps = pps.tile([1, QC], F32, tag="pp", name=f"bps{tagp}{half}")
                for kc in range(n_kc):
                    nc.tensor.matmul(ps[:, :], btcol[:, kc:kc + 1],
                                     tiles[kc][:, half * QC:(half + 1) * QC],
                                     start=(kc == 0), stop=(kc == n_kc - 1))
                nc.vector.tensor_tensor(out=bsum[:, half * QC:(half + 1) * QC],
                                        in0=ps[:, :],
                                        in1=brow[:, half * QC:(half + 1) * QC],
                                        op=OP.add)
            bd = scr_pool.tile([1, CQ], F32, tag=f"bd{tagp}", name=f"bd{tagp}")
            nc.gpsimd.dma_start(out=bd[:, :], in_=bsum[:, :])
            return bd

        # ---- query side, chunked by 512 tokens ----
        wq = load_w(t["Wq"], KC_Q, "q", gq)
        bdq = bias_fold(wq, "bq", KC_Q, btq8, "q", scrb)
        bdqap = bdq[0:1, :]
        nc.scalar.dma_start(
            out=bq_cols[:, :],
            in_=bass.AP(tensor=bdqap.tensor, offset=bdqap.offset,
                        ap=[[1, 128], [128, KC_Q]]))
        for t2 in range(NQ2):
            xqT = [xTp.tile([128, QC], BF16, tag=f"xqT{kc}",
                            name=f"xqT{kc}_{t2}") for kc in range(KC_Q)]
            for i in range(4):
                ln_tile(t["xq"], t2 * 4 + i, CQ, 2, 512, xqT, i * 128)
            for oc in range(KC_Q):
                ps = pps.tile([128, QC], F32, tag="pp", name=f"psq{oc}_{t2}")
                for kc in range(KC_Q):
                    nc.tensor.matmul(ps[:, :],
                                     wq[kc][:, oc * 128:(oc + 1) * 128],
                                     xqT[kc][:, :],
                                     start=(kc == 0), stop=(kc == KC_Q - 1))
                nc.scalar.activation(
                    out=qTc[oc][:, t2 * QC:(t2 + 1) * QC], in_=ps[:, :],
                    func=AF.Identity, bias=bq_cols[:, oc:oc + 1], scale=1.0)

        # ---- context side, chunked by 512 tokens ----
        wk = load_w(t["Wk"], KC_C, "k", gc)
        wv = load_w(t["Wv"], KC_C, "v", gc)
        bdk = bias_fold(wk, "bk", KC_C, btc8, "k", scrb)
        bdkap = bdk[0:1, :]
        nc.scalar.dma_start(
            out=bk_cols[:, :],
            in_=bass.AP(tensor=bdkap.tensor, offset=bdkap.offset,
                        ap=[[1, 128], [128, KC_Q]]))
        bdv = bias_fold(wv, "bv", KC_C, btc8, "v", scrb)
        bdvap = bdv[0:1, :]
        nc.gpsimd.dma_start(
            out=bvb[:, :],
            in_=bass.AP(tensor=bdvap.tensor, offset=bdvap.offset,
                        ap=[[0, 128], [1, CQ]]))
        for t4 in range(NK // QC):
            xcT = [xTp.tile([128, QC], BF16, tag=f"xcT{kc}",
                            name=f"xcT{kc}_{t4}") for kc in range(KC_C)]
            for i in range(4):
                ln_tile(t["xc"], t4 * 4 + i, CK, 3, 256, xcT, i * 128)
            for oc in range(KC_Q):
                ps = pps.tile([128, QC], F32, tag="pp", name=f"psk{oc}_{t4}")
                for kc in range(KC_C):
                    nc.tensor.matmul(ps[:, :],
                                     wk[kc][:, oc * 128:(oc + 1) * 128],
                                     xcT[kc][:, :],
                                     start=(kc == 0), stop=(kc == KC_C - 1))
                nc.scalar.activation(
                    out=kTc[oc][t4][:, :], in_=ps[:, :], func=AF.Identity,
                    bias=bk_cols[:, oc:oc + 1], scale=1.0)
            for ki in range(4):
                kt = t4 * 4 + ki
                for v2 in range(CQ // QC):
                    ps = pps.tile([128, QC], F32, tag="pp",
                                  name=f"psv{kt}_{v2}")
                    for kc in range(KC_C):
                        nc.tensor.matmul(ps[:, :],
                                         xcT[kc][:, ki * 128:(ki + 1) * 128],
                                         wv[kc][:, v2 * QC:(v2 + 1) * QC],
                                         start=(kc == 0), stop=(kc == KC_C - 1))
                    nc.vector.tensor_tensor(
                        out=v_g[t4][:, ki, v2 * 8:(v2 + 1) * 8, 0:D],
                        in0=ps[:, :].rearrange("p (h d) -> p h d", d=D),
                        in1=bvb[:, v2 * QC:(v2 + 1) * QC].rearrange(
                            "p (h d) -> p h d", d=D),
                        op=OP.add)
                nc.vector.memset(v_g[t4][:, ki, :, D:D + 1], 1.0)

    # ---------------- phase 3: attention ----------------
    if "attn" not in stages:
        # timing-only partial build: flush something derived to out
        with tc.tile_pool(name="fl", bufs=1) as fl:
            fb = fl.tile([128, QC], F32, name="fb")
            nc.vector.tensor_copy(out=fb[:, :], in_=qTc[0][:, 0:QC])
            nc.sync.dma_start(out=out.ap()[0:128, 0:QC], in_=fb[:, :])
        es.close()
        return
    late = es.enter_context(tc.tile_pool(name="late", bufs=1))
    attT = late.tile([128, KC_Q, NQ], BF16, name="attT")
    wo = late.tile([128, KC_Q, CQ], BF16, name="wo")
    bob = late.tile([128, CQ], F32, name="bob")

    with tc.tile_pool(name="scps", bufs=2, space="PSUM") as scps, \
         tc.tile_pool(name="attps", bufs=2, space="PSUM") as attps, \
         tc.tile_pool(name="ep", bufs=4) as ep, \
         tc.tile_pool(name="rp", bufs=4) as rp, \
         tc.tile_pool(name="tmp1", bufs=2) as tmp1p, \
         tc.tile_pool(name="scr", bufs=4, space="DRAM") as scr, \
         tc.tile_pool(name="wfp2", bufs=2) as wfp2:

        nc.gpsimd.dma_start(out=bob[:, :], in_=_bcast_ap(t["bo"], 128, CQ))
        for kc in range(KC_Q):
            wof = wfp2.tile([128, CQ], F32, tag="wof", name=f"wof{kc}")
            nc.scalar.dma_start(out=wof[:, :],
                                in_=t["Wo"].ap()[kc * 128:(kc + 1) * 128, :])
            nc.gpsimd.tensor_copy(out=wo[:, kc, :], in_=wof[:, :])

        for hp in range(H // 2):
            att = {}
            for par in range(2):
                h = 2 * hp + par
                att[par] = attps.tile([D + 1, NQ], F32, tag="att",
                                      name=f"attp{h}")
            for kt in range(NKT):
                for par in range(2):
                    h, lo = 2 * hp + par, par * 64
                    sc = scps.tile([128, NQ], F32, tag="sc",
                                   name=f"sc{h}_{kt}")
                    for q2 in range(NQ2):
                        nc.tensor.matmul(
                            sc[:, q2 * QC:(q2 + 1) * QC],
                            kTc[hp][kt // 4][lo:lo + 64,
                                             (kt % 4) * 128:(kt % 4 + 1) * 128],
                            qTc[hp][lo:lo + 64, q2 * QC:(q2 + 1) * QC],
                            start=True, stop=True)
                    # one exp over the full q width (both psum banks):
                    # halves the per-instruction ACT overhead
                    e = ep.tile([128, NQ], BF16, tag="e", name=f"e{h}_{kt}")
                    nc.scalar.activation(out=e[:, :], in_=sc[:, :],
                                         func=AF.Exp, scale=SM_SCALE)
                    # attended + softmax denominator in one matmul:
                    # lhsT = [V_h | ones], row 64 of psum = sum(exp)
                    for q2 in range(NQ2):
                        nc.tensor.matmul(
                            att[par][:, q2 * QC:(q2 + 1) * QC],
                            v_g[kt // 4][:, kt % 4, h, :],
                            e[:, q2 * QC:(q2 + 1) * QC],
                            start=(kt == 0), stop=(kt == NKT - 1))
            for par in range(2):
                h = 2 * hp + par
                # drain psum to SBUF right away so the accumulator slot
                # frees for the next head pair; the (slow) normalize chain
                # then runs off the SBUF copy, off the critical path
                atc = rp.tile([64, NQ], F32, tag="atc", name=f"atc{h}")
                nc.vector.tensor_copy(out=atc[:, :], in_=att[par][0:D, :])
                rec = rp.tile([65, NQ], F32, tag="rec", name=f"rec{h}")
                nc.vector.reciprocal(out=rec[64:65, :], in_=att[par][64:65, :])
                sd = scr.tile([1, NQ], F32, tag="sd", name=f"sd{h}")
                nc.sync.dma_start(out=sd[:, :], in_=rec[64:65, :])
                rb = rp.tile([64, NQ], F32, tag="rb", name=f"rb{h}")
                nc.sync.dma_start(
                    out=rb[:, :],
                    in_=bass.AP(tensor=sd.tensor, offset=sd.offset,
                                ap=[[0, 64], [1, NQ]]))
                if par == 0:
                    nc.vector.tensor_mul(out=attT[0:64, hp, :],
                                         in0=atc[:, :], in1=rb[:, :])
                else:
                    # odd head: normalize at partitions 0-63, then DMA
                    # shifts it to partitions 64-127 of the attT chunk
                    tm = tmp1p.tile([64, NQ], BF16, tag="tm", name=f"tm{h}")
                    nc.vector.tensor_mul(out=tm[:, :],
                                         in0=atc[:, :], in1=rb[:, :])
                    nc.sync.dma_start(out=attT[64:128, hp, :], in_=tm[:, :])

    # ---------------- phase 4: out projection ----------------
    with tc.tile_pool(name="ops", bufs=2, space="PSUM") as ops, \
         tc.tile_pool(name="op", bufs=2) as op_pool:
        if "out" not in stages:
            fb2 = op_pool.tile([128, QC], F32, name="fb2")
            nc.vector.tensor_copy(out=fb2[:, :], in_=attT[:, 0, 0:QC])
            nc.sync.dma_start(out=out.ap()[0:128, 0:QC], in_=fb2[:, :])
        for qt in range(NQT if "out" in stages else 0):
            osb = op_pool.tile([128, CQ], F32, tag="osb", name=f"osb{qt}")
            for cc in range(CQ // QC):
                ps = ops.tile([128, QC], F32, tag="opp", name=f"pso{qt}_{cc}")
                for kc in range(KC_Q):
                    nc.tensor.matmul(
                        ps[:, :],
                        attT[:, kc, qt * 128:(qt + 1) * 128],
                        wo[:, kc, cc * QC:(cc + 1) * QC],
                        start=(kc == 0), stop=(kc == KC_Q - 1))
                nc.vector.tensor_tensor(out=osb[:, cc * QC:(cc + 1) * QC],
                                        in0=ps[:, :],
                                        in1=bob[:, cc * QC:(cc + 1) * QC],
                                        op=OP.add)
            nc.sync.dma_start(out=out.ap()[qt * 128:(qt + 1) * 128, :],
                              in_=osb[:, :])

    es.close()


def build(split_waits=True):
    nc = bass.Bass("TRN2", target_bir_lowering=False, debug=False,
                   num_devices=N_CORES)
    t = {
        "xq": nc.dram_tensor("xq", [NQ, CQ], F32, kind="ExternalInput"),
        "xc": nc.dram_tensor("xc", [NK, CK], F32, kind="ExternalInput"),
        "Wq": nc.dram_tensor("Wq", [CQ, CQ], F32, kind="ExternalInput"),
        "Wk": nc.dram_tensor("Wk", [CK, CQ], F32, kind="ExternalInput"),
        "Wv": nc.dram_tensor("Wv", [CK, CQ], F32, kind="ExternalInput"),
        "Wo": nc.dram_tensor("Wo", [CQ, CQ], F32, kind="ExternalInput"),
        "bq": nc.dram_tensor("bq", [CQ], F32, kind="ExternalInput"),
        "bk": nc.dram_tensor("bk", [CQ], F32, kind="ExternalInput"),
        "bv": nc.dram_tensor("bv", [CQ], F32, kind="ExternalInput"),
        "bo": nc.dram_tensor("bo", [CQ], F32, kind="ExternalInput"),
        "gamma_q": nc.dram_tensor("gamma_q", [CQ], F32, kind="ExternalInput"),
        "beta_q": nc.dram_tensor("beta_q", [CQ], F32, kind="ExternalInput"),
        "gamma_ctx": nc.dram_tensor("gamma_ctx", [CK], F32, kind="ExternalInput"),
        "beta_ctx": nc.dram_tensor("beta_ctx", [CK], F32, kind="ExternalInput"),
    }
    out = nc.dram_tensor("out", [NQ, CQ], F32, kind="ExternalOutput")
    with tile.TileContext(nc) as tc:
        _emit(tc, t, out)
    if split_waits:
        _split_excess_waits(nc)
    return nc


_NC = None


def _in_maps(inputs):
    q = np.ascontiguousarray(np.asarray(inputs["query_tokens"], dtype=np.float32))
    c = np.ascontiguousarray(np.asarray(inputs["context_tokens"], dtype=np.float32))
    shared = {k: np.ascontiguousarray(np.asarray(inputs[k], dtype=np.float32))
              for k in ("Wq", "Wk", "Wv", "Wo", "bq", "bk", "bv", "bo",
                        "gamma_q", "beta_q", "gamma_ctx", "beta_ctx")}
    maps = []
    for core in range(N_CORES):
        b, half = core // 2, core % 2
        m = dict(shared)
        m["xq"] = np.ascontiguousarray(q[b, half * NQ:(half + 1) * NQ, :])
        m["xc"] = np.ascontiguousarray(c[b])
        maps.append(m)
    return maps


def run_sharded(inputs, **kwargs):
    global _NC
    if _NC is None:
        _NC = build()
    return run_bass_kernel_spmd(_NC, _in_maps(inputs),
                                core_ids=list(range(N_CORES)), **kwargs)


def kernel(**inputs) -> np.ndarray:
    res = run_sharded(inputs)
    out = np.empty((B, NQ_FULL, CQ), np.float32)
    for core in range(N_CORES):
        b, half = core // 2, core % 2
        out[b, half * NQ:(half + 1) * NQ, :] = res.results[core]["out"]
    return out



# revision 10
# speedup vs baseline: 1.7312x; 1.7312x over previous
"""Trainium2 Bass kernel for CrossAttention (LN -> QKV proj -> MHA -> out proj).

Sharding: data-parallel over (batch, query-half): 8 shards for B=4.
Each core gets a [1024, 1024] query-token slice and the full [2048, 768]
context for its batch, and produces a [1024, 1024] output slice.

Per-core dataflow (matmul operands bf16, accumulation fp32 in PSUM):
  - LayerNorm on query/context tokens in natural [tok, C] layout (DVE
    bn_stats / bn_aggr), gamma/beta applied with partition-broadcast rows.
  - Transpose LN'd activations to channel-major [C, tok] via DMA xbar
    transpose (bf16) so channels sit on the contraction (partition) axis.
    Activations/projections are chunked (512 tokens) so projections start
    while later chunks are still normalizing.
  - QT = Wq^T @ XqT, KT = Wk^T @ XcT (channel-major), V natural [tok, C].
  - Attention runs per head-quad: scoresT[k, q] = K_h @ Q_h^T (contraction
    D=64; even/odd heads at partitions 0-63/64-127 land on disjoint PE row
    groups and overlap), exp on ACT straight out of PSUM (scores are
    bounded, so no max subtraction).  attendedT accumulates per head pair
    into one [128, 512] psum via column tiling (head0 rows 0-63, head1
    rows 64-127, concurrent).  Softmax denominators come from ones-vector
    matmuls, 4 heads packed at output partitions 0/32/64/96 of one psum.
  - Normalize with DVE reciprocal + DRAM-bounce partition broadcast, then
    out = attendedT^T @ Wo + bo in natural layout, DMA out.
"""

import numpy as np

import concourse.bass as bass
import concourse.tile as tile
from concourse import mybir
from concourse.bass_utils import run_bass_kernel_spmd

F32 = mybir.dt.float32
BF16 = mybir.dt.bfloat16
AF = mybir.ActivationFunctionType
OP = mybir.AluOpType

B, NQ_FULL, NK, CQ, CK, H, D = 4, 2048, 2048, 1024, 768, 16, 64
NQ = 1024            # per-core query tokens
N_CORES = 8
EPS = 1e-5
SM_SCALE = 1.0 / np.sqrt(D)  # 0.125

KC_Q = CQ // 128     # 8  contraction chunks for CQ
KC_C = CK // 128     # 6  contraction chunks for CK
NQT = NQ // 128      # 8  query token tiles
NKT = NK // 128      # 16 context token tiles
QC = 512             # q processed in chunks of 512 (psum free-dim limit)
NQ2 = NQ // QC       # 2


def _split_excess_waits(nc, max_waits=1):
    """walrus in this container accepts at most one sync wait per
    instruction; Tile's kernel-tail drain carries several.  Hoist excess
    waits onto single-wait NOPs that precede the instruction on the same
    engine (absolute sem waits commute, so this is semantics-preserving)."""
    for fn in nc.m.functions:
        for blk in fn.blocks:
            out = []
            dirty = False
            for inst in list(blk.instructions):
                si = inst.sync_info
                if si is not None and len(si.on_wait) > max_waits:
                    waits = list(si.on_wait)
                    for k, w in enumerate(waits[:-max_waits]):
                        nop = mybir.InstNoOp(
                            name=f"wsplit-{inst.name}-{k}", ins=[], outs=[])
                        nop.engine = inst.engine
                        nop.sync_info = mybir.SyncInfo(on_wait=[w], on_update=[])
                        out.append(nop)
                    inst.sync_info = mybir.SyncInfo(
                        on_wait=waits[-max_waits:], on_update=list(si.on_update))
                    dirty = True
                out.append(inst)
            if dirty:
                blk.instructions = out


def _bcast_ap(handle, n_parts, n_free):
    """DRAM [n_free] vector replicated across n_parts partitions."""
    return bass.AP(tensor=handle.ap().tensor, offset=0,
                   ap=[[0, n_parts], [1, n_free]])


def _emit(tc, t, out, stages=("proj", "attn", "out"), variant=()):
    from contextlib import ExitStack
    nc = tc.nc

    es = ExitStack()
    persist = es.enter_context(tc.tile_pool(name="persist", bufs=1))

    # chunked persistent tensors (distinct tags => distinct slots)
    # kT split per (channel-chunk, token-chunk): finer deps let attention
    # start while later context chunks are still projecting
    qTc = [persist.tile([128, NQ], BF16, tag=f"qT{oc}", name=f"qT{oc}")
           for oc in range(KC_Q)]
    kTc = [[persist.tile([128, QC], BF16, tag=f"kT{oc}_{t4}",
                         name=f"kT{oc}_{t4}") for t4 in range(NK // QC)]
           for oc in range(KC_Q)]
    # V with a ones column appended per head: the attended-value matmul
    # then also emits the softmax denominator (row 64 of its psum)
    v_g = [persist.tile([128, 4, H, D + 1], BF16, tag=f"v{g}", name=f"v{g}")
           for g in range(NKT // 4)]
    bq_cols = persist.tile([128, KC_Q], F32)
    bk_cols = persist.tile([128, KC_Q], F32)
    bvb = persist.tile([128, CQ], F32)
    eps_t = persist.tile([128, 1], F32)

    nc.vector.memset(eps_t[:, :], EPS)

    # ---------------- phase 1+2: LN, transpose, projections ----------------
    with tc.tile_pool(name="pps", bufs=3, space="PSUM") as pps, \
         tc.tile_pool(name="lnw", bufs=1) as lnw, \
         tc.tile_pool(name="xfp", bufs=3) as xfp, \
         tc.tile_pool(name="stp", bufs=4) as stp, \
         tc.tile_pool(name="bfp", bufs=4) as bfp, \
         tc.tile_pool(name="wfp", bufs=3) as wfp, \
         tc.tile_pool(name="xTp", bufs=2) as xTp, \
         tc.tile_pool(name="wbp", bufs=1) as wbp, \
         tc.tile_pool(name="scrb", bufs=1, space="DRAM") as scrb:

        # gamma/beta as [128, n_chunk] columns; LN fold:
        # LN(x)@W + b = ((x-mu)*rstd) @ (gamma.W) + (b + (beta/gamma)@(gamma.W))
        gq = lnw.tile([128, KC_Q], F32, name="gq")
        gc = lnw.tile([128, KC_C], F32, name="gc")
        btq = lnw.tile([128, KC_Q], F32, name="btq")
        btc = lnw.tile([128, KC_C], F32, name="btc")
        nc.gpsimd.dma_start(out=gq[:, :],
                            in_=t["gamma_q"].ap().rearrange("(j p) -> p j", p=128))
        nc.gpsimd.dma_start(out=gc[:, :],
                            in_=t["gamma_ctx"].ap().rearrange("(j p) -> p j", p=128))
        nc.gpsimd.dma_start(out=btq[:, :],
                            in_=t["beta_q"].ap().rearrange("(j p) -> p j", p=128))
        nc.gpsimd.dma_start(out=btc[:, :],
                            in_=t["beta_ctx"].ap().rearrange("(j p) -> p j", p=128))
        btq8 = lnw.tile([128, KC_Q], BF16, name="btq8")
        btc8 = lnw.tile([128, KC_C], BF16, name="btc8")
        rg = lnw.tile([128, KC_Q], F32, name="rg")
        nc.vector.reciprocal(out=rg[:, :KC_Q], in_=gq[:, :])
        nc.vector.tensor_mul(out=btq8[:, :], in0=btq[:, :], in1=rg[:, :KC_Q])
        nc.vector.reciprocal(out=rg[:, :KC_C], in_=gc[:, :])
        nc.vector.tensor_mul(out=btc8[:, :], in0=btc[:, :], in1=rg[:, :KC_C])

        def ln_tile(x_dram, i, C, n_sub, sub, xT3, col0):
            """LN one [128, C] token tile (gamma/beta folded into the
            weights; input DMA-cast to bf16 on load so DVE runs in its
            2x/4x packed modes), then one fused xbar transpose writes all
            C//128 channel chunks of xT3 (out[p, j, t] = x[t, j*128+p])."""
            xb = xfp.tile([128, C], BF16, tag="xf", name=f"xf_{i}_{C}")
            nc.gpsimd.dma_start(out=xb[:, :],
                                in_=x_dram.ap()[i * 128:(i + 1) * 128, :])
            xbf = bfp.tile([128, C], BF16, tag="xbf", name=f"xbf_{i}_{C}")
            if "fast_ln" in variant:  # timing probe only: skip LN math
                nc.vector.tensor_copy(out=xbf[:, :], in_=xb[:, :])
            else:
                st = stp.tile([128, n_sub, 6], F32, tag="st",
                              name=f"st_{i}_{C}")
                for s in range(n_sub):
                    nc.vector.bn_stats(out=st[:, s, :],
                                       in_=xb[:, s * sub:(s + 1) * sub])
                mv = stp.tile([128, 2], F32, tag="mv", name=f"mv_{i}_{C}")
                nc.vector.bn_aggr(out=mv[:, :], in_=st[:, :, :])
                nc.scalar.activation(out=mv[:, 1:2], in_=mv[:, 1:2],
                                     func=AF.Sqrt, bias=eps_t[:, :], scale=1.0)
                nc.vector.reciprocal(out=mv[:, 1:2], in_=mv[:, 1:2])
                nc.vector.tensor_scalar(out=xbf[:, :], in0=xb[:, :],
                                        scalar1=mv[:, 0:1], scalar2=mv[:, 1:2],
                                        op0=OP.subtract, op1=OP.mult)
            if "no_xpose" in variant:  # timing probe: plain copy, no xbar
                nc.sync.dma_start(
                    out=xT3[:, :, col0:col0 + 128],
                    in_=xbf[:, :].rearrange("p (j t) -> p j t", t=128))
                return
            nc.sync.dma_start(out=xT3[:, :, col0:col0 + 128],
                              in_=xbf[:, :], transpose=True)

        def load_w(dram, n_chunks, tagp, gcol):
            tiles = []
            for kc in range(n_chunks):
                wf = wfp.tile([128, CQ], BF16, tag="wf", name=f"wf{tagp}{kc}")
                nc.gpsimd.dma_start(out=wf[:, :],
                                    in_=dram.ap()[kc * 128:(kc + 1) * 128, :])
                wb = wbp.tile([128, CQ], BF16, tag=f"w{tagp}{kc}",
                              name=f"w{tagp}{kc}")
                nc.scalar.activation(out=wb[:, :], in_=wf[:, :],
                                     func=AF.Copy, scale=gcol[:, kc:kc + 1])
                tiles.append(wb)
            return tiles

        def bias_fold(tiles, bname, n_kc, btcol, tagp, scr_pool):
            """bias' = b + beta~ @ W' -> DRAM row [CQ]."""
            brow = lnw.tile([1, CQ], F32, tag="brow",
                            name=f"brow{tagp}")
            nc.gpsimd.dma_start(out=brow[:, :],
                                in_=bass.AP(tensor=t[bname].ap().tensor,
                                            offset=0, ap=[[0, 1], [1, CQ]]))
            bsum = lnw.tile([1, CQ], F32, tag="bsum",
                            name=f"bsum{tagp}")
            for half in range(2):
                ps = pps.tile([1, QC], F32, tag="pp", name=f"bps{tagp}{half}")
                for kc in range(n_kc):
                    nc.tensor.matmul(ps[:, :], btcol[:, kc:kc + 1],
                                     tiles[kc][:, half * QC:(half + 1) * QC],
                                     start=(kc == 0), stop=(kc == n_kc - 1))
                nc.vector.tensor_tensor(out=bsum[:, half * QC:(half + 1) * QC],
                                        in0=ps[:, :],
                                        in1=brow[:, half * QC:(half + 1) * QC],
                                        op=OP.add)
            bd = scr_pool.tile([1, CQ], F32, tag=f"bd{tagp}", name=f"bd{tagp}")
            nc.gpsimd.dma_start(out=bd[:, :], in_=bsum[:, :])
            return bd

        # ---- query side, chunked by 512 tokens ----
        wq = load_w(t["Wq"], KC_Q, "q", gq)
        bdq = bias_fold(wq, "bq", KC_Q, btq8, "q", scrb)
        bdqap = bdq[0:1, :]
        nc.scalar.dma_start(
            out=bq_cols[:, :],
            in_=bass.AP(tensor=bdqap.tensor, offset=bdqap.offset,
                        ap=[[1, 128], [128, KC_Q]]))
        for t2 in range(NQ2):
            xqT = xTp.tile([128, KC_Q, QC], BF16, tag="xqT",
                           name=f"xqT_{t2}")
            for i in range(4):
                ln_tile(t["xq"], t2 * 4 + i, CQ, 2, 512, xqT, i * 128)
            for oc in range(KC_Q):
                ps = pps.tile([128, QC], F32, tag="pp", name=f"psq{oc}_{t2}")
                for kc in range(KC_Q):
                    nc.tensor.matmul(ps[:, :],
                                     wq[kc][:, oc * 128:(oc + 1) * 128],
                                     xqT[:, kc, :],
                                     start=(kc == 0), stop=(kc == KC_Q - 1))
                nc.vector.tensor_scalar_add(
                    out=qTc[oc][:, t2 * QC:(t2 + 1) * QC], in0=ps[:, :],
                    scalar1=bq_cols[:, oc:oc + 1])

        # ---- context side, chunked by 512 tokens ----
        wk = load_w(t["Wk"], KC_C, "k", gc)
        wv = load_w(t["Wv"], KC_C, "v", gc)
        bdk = bias_fold(wk, "bk", KC_C, btc8, "k", scrb)
        bdkap = bdk[0:1, :]
        nc.scalar.dma_start(
            out=bk_cols[:, :],
            in_=bass.AP(tensor=bdkap.tensor, offset=bdkap.offset,
                        ap=[[1, 128], [128, KC_Q]]))
        bdv = bias_fold(wv, "bv", KC_C, btc8, "v", scrb)
        bdvap = bdv[0:1, :]
        nc.gpsimd.dma_start(
            out=bvb[:, :],
            in_=bass.AP(tensor=bdvap.tensor, offset=bdvap.offset,
                        ap=[[0, 128], [1, CQ]]))
        for t4 in range(NK // QC):
            xcT = xTp.tile([128, KC_C, QC], BF16, tag="xcT",
                           name=f"xcT_{t4}")
            for i in range(4):
                ln_tile(t["xc"], t4 * 4 + i, CK, 3, 256, xcT, i * 128)
            for oc in range(KC_Q):
                ps = pps.tile([128, QC], F32, tag="pp", name=f"psk{oc}_{t4}")
                for kc in range(KC_C):
                    nc.tensor.matmul(ps[:, :],
                                     wk[kc][:, oc * 128:(oc + 1) * 128],
                                     xcT[:, kc, :],
                                     start=(kc == 0), stop=(kc == KC_C - 1))
                nc.vector.tensor_scalar_add(
                    out=kTc[oc][t4][:, :], in0=ps[:, :],
                    scalar1=bk_cols[:, oc:oc + 1])
            for ki in range(4):
                kt = t4 * 4 + ki
                for v2 in range(CQ // QC):
                    ps = pps.tile([128, QC], F32, tag="pp",
                                  name=f"psv{kt}_{v2}")
                    for kc in range(KC_C):
                        nc.tensor.matmul(ps[:, :],
                                         xcT[:, kc, ki * 128:(ki + 1) * 128],
                                         wv[kc][:, v2 * QC:(v2 + 1) * QC],
                                         start=(kc == 0), stop=(kc == KC_C - 1))
                    nc.vector.tensor_tensor(
                        out=v_g[t4][:, ki, v2 * 8:(v2 + 1) * 8, 0:D],
                        in0=ps[:, :].rearrange("p (h d) -> p h d", d=D),
                        in1=bvb[:, v2 * QC:(v2 + 1) * QC].rearrange(
                            "p (h d) -> p h d", d=D),
                        op=OP.add)
                nc.vector.memset(v_g[t4][:, ki, :, D:D + 1], 1.0)

    # ---------------- phase 3: attention ----------------
    if "attn" not in stages:
        # timing-only partial build: flush something derived to out
        with tc.tile_pool(name="fl", bufs=1) as fl:
            fb = fl.tile([128, QC], F32, name="fb")
            nc.vector.tensor_copy(out=fb[:, :], in_=qTc[0][:, 0:QC])
            nc.sync.dma_start(out=out.ap()[0:128, 0:QC], in_=fb[:, :])
        es.close()
        return
    late = es.enter_context(tc.tile_pool(name="late", bufs=1))
    attT = late.tile([128, KC_Q, NQ], BF16, name="attT")
    wo = late.tile([128, KC_Q, CQ], BF16, name="wo")
    bob = late.tile([128, CQ], F32, name="bob")

    with tc.tile_pool(name="scps", bufs=2, space="PSUM") as scps, \
         tc.tile_pool(name="attps", bufs=2, space="PSUM") as attps, \
         tc.tile_pool(name="ep", bufs=4) as ep, \
         tc.tile_pool(name="rp", bufs=4) as rp, \
         tc.tile_pool(name="tmp1", bufs=2) as tmp1p, \
         tc.tile_pool(name="scr", bufs=4, space="DRAM") as scr:

        nc.gpsimd.dma_start(out=bob[:, :], in_=_bcast_ap(t["bo"], 128, CQ))
        for kc in range(KC_Q):
            nc.gpsimd.dma_start(out=wo[:, kc, :],
                                in_=t["Wo"].ap()[kc * 128:(kc + 1) * 128, :])

        for hp in range(H // 2):
            att = {}
            for par in range(2):
                h = 2 * hp + par
                att[par] = attps.tile([D + 1, NQ], F32, tag="att",
                                      name=f"attp{h}")

            def att_mm(kt, es):
                # attended + softmax denominator in one matmul:
                # lhsT = [V_h | ones], row 64 of psum = sum(exp)
                for par in range(2):
                    for q2 in range(NQ2):
                        nc.tensor.matmul(
                            att[par][:, q2 * QC:(q2 + 1) * QC],
                            v_g[kt // 4][:, kt % 4, 2 * hp + par, :],
                            es[par][:, q2 * QC:(q2 + 1) * QC],
                            start=(kt == 0), stop=(kt == NKT - 1))

            # software-pipelined: emit scores+exp for kt before the
            # attended matmuls of kt-1, so the (in-order) PE queue never
            # blocks on ACT's exp — sc(kt+1) runs while exp(kt) is busy,
            # and ACT stays saturated (it is the bottleneck here).
            pend = None
            for kt in range(NKT):
                cur = []
                for par in range(2):
                    h, lo = 2 * hp + par, par * 64
                    sc = scps.tile([128, NQ], F32, tag="sc",
                                   name=f"sc{h}_{kt}")
                    for q2 in range(NQ2):
                        nc.tensor.matmul(
                            sc[:, q2 * QC:(q2 + 1) * QC],
                            kTc[hp][kt // 4][lo:lo + 64,
                                             (kt % 4) * 128:(kt % 4 + 1) * 128],
                            qTc[hp][lo:lo + 64, q2 * QC:(q2 + 1) * QC],
                            start=True, stop=True)
                    # one exp over the full q width (both psum banks):
                    # halves the per-instruction ACT overhead
                    e = ep.tile([128, NQ], BF16, tag="e", name=f"e{h}_{kt}")
                    nc.scalar.activation(out=e[:, :], in_=sc[:, :],
                                         func=AF.Exp, scale=SM_SCALE)
                    cur.append(e)
                if pend is not None:
                    att_mm(pend[0], pend[1])
                pend = (kt, cur)
            att_mm(pend[0], pend[1])
            for par in range(2):
                h = 2 * hp + par
                # drain psum to SBUF right away so the accumulator slot
                # frees for the next head pair; the (slow) normalize chain
                # then runs off the SBUF copy, off the critical path
                atc = rp.tile([64, NQ], F32, tag="atc", name=f"atc{h}")
                nc.vector.tensor_copy(out=atc[:, :], in_=att[par][0:D, :])
                rec = rp.tile([65, NQ], F32, tag="rec", name=f"rec{h}")
                nc.vector.reciprocal(out=rec[64:65, :], in_=att[par][64:65, :])
                sd = scr.tile([1, NQ], F32, tag="sd", name=f"sd{h}")
                nc.sync.dma_start(out=sd[:, :], in_=rec[64:65, :])
                rb = rp.tile([64, NQ], F32, tag="rb", name=f"rb{h}")
                nc.sync.dma_start(
                    out=rb[:, :],
                    in_=bass.AP(tensor=sd.tensor, offset=sd.offset,
                                ap=[[0, 64], [1, NQ]]))
                if par == 0:
                    nc.vector.tensor_mul(out=attT[0:64, hp, :],
                                         in0=atc[:, :], in1=rb[:, :])
                else:
                    # odd head: normalize at partitions 0-63, then DMA
                    # shifts it to partitions 64-127 of the attT chunk
                    tm = tmp1p.tile([64, NQ], BF16, tag="tm", name=f"tm{h}")
                    nc.vector.tensor_mul(out=tm[:, :],
                                         in0=atc[:, :], in1=rb[:, :])
                    nc.sync.dma_start(out=attT[64:128, hp, :], in_=tm[:, :])

    # ---------------- phase 4: out projection ----------------
    with tc.tile_pool(name="ops", bufs=2, space="PSUM") as ops, \
         tc.tile_pool(name="op", bufs=2) as op_pool:
        if "out" not in stages:
            fb2 = op_pool.tile([128, QC], F32, name="fb2")
            nc.vector.tensor_copy(out=fb2[:, :], in_=attT[:, 0, 0:QC])
            nc.sync.dma_start(out=out.ap()[0:128, 0:QC], in_=fb2[:, :])
        for qt in range(NQT if "out" in stages else 0):
            osb = op_pool.tile([128, CQ], F32, tag="osb", name=f"osb{qt}")
            for cc in range(CQ // QC):
                ps = ops.tile([128, QC], F32, tag="opp", name=f"pso{qt}_{cc}")
                for kc in range(KC_Q):
                    nc.tensor.matmul(
                        ps[:, :],
                        attT[:, kc, qt * 128:(qt + 1) * 128],
                        wo[:, kc, cc * QC:(cc + 1) * QC],
                        start=(kc == 0), stop=(kc == KC_Q - 1))
                nc.vector.tensor_tensor(out=osb[:, cc * QC:(cc + 1) * QC],
                                        in0=ps[:, :],
                                        in1=bob[:, cc * QC:(cc + 1) * QC],
                                        op=OP.add)
            nc.sync.dma_start(out=out.ap()[qt * 128:(qt + 1) * 128, :],
                              in_=osb[:, :])

    es.close()


def build(split_waits=True):
    nc = bass.Bass("TRN2", target_bir_lowering=False, debug=False,
                   num_devices=N_CORES)
    t = {
        "xq": nc.dram_tensor("xq", [NQ, CQ], F32, kind="ExternalInput"),
        "xc": nc.dram_tensor("xc", [NK, CK], F32, kind="ExternalInput"),
        "Wq": nc.dram_tensor("Wq", [CQ, CQ], F32, kind="ExternalInput"),
        "Wk": nc.dram_tensor("Wk", [CK, CQ], F32, kind="ExternalInput"),
        "Wv": nc.dram_tensor("Wv", [CK, CQ], F32, kind="ExternalInput"),
        "Wo": nc.dram_tensor("Wo", [CQ, CQ], F32, kind="ExternalInput"),
        "bq": nc.dram_tensor("bq", [CQ], F32, kind="ExternalInput"),
        "bk": nc.dram_tensor("bk", [CQ], F32, kind="ExternalInput"),
        "bv": nc.dram_tensor("bv", [CQ], F32, kind="ExternalInput"),
        "bo": nc.dram_tensor("bo", [CQ], F32, kind="ExternalInput"),
        "gamma_q": nc.dram_tensor("gamma_q", [CQ], F32, kind="ExternalInput"),
        "beta_q": nc.dram_tensor("beta_q", [CQ], F32, kind="ExternalInput"),
        "gamma_ctx": nc.dram_tensor("gamma_ctx", [CK], F32, kind="ExternalInput"),
        "beta_ctx": nc.dram_tensor("beta_ctx", [CK], F32, kind="ExternalInput"),
    }
    out = nc.dram_tensor("out", [NQ, CQ], F32, kind="ExternalOutput")
    with tile.TileContext(nc) as tc:
        _emit(tc, t, out)
    if split_waits:
        _split_excess_waits(nc)
    return nc


_NC = None


def _in_maps(inputs):
    q = np.ascontiguousarray(np.asarray(inputs["query_tokens"], dtype=np.float32))
    c = np.ascontiguousarray(np.asarray(inputs["context_tokens"], dtype=np.float32))
    shared = {k: np.ascontiguousarray(np.asarray(inputs[k], dtype=np.float32))
              for k in ("Wq", "Wk", "Wv", "Wo", "bq", "bk", "bv", "bo",
                        "gamma_q", "beta_q", "gamma_ctx", "beta_ctx")}
    maps = []
    for core in range(N_CORES):
        b, half = core // 2, core % 2
        m = dict(shared)
        m["xq"] = np.ascontiguousarray(q[b, half * NQ:(half + 1) * NQ, :])
        m["xc"] = np.ascontiguousarray(c[b])
        maps.append(m)
    return maps


def run_sharded(inputs, **kwargs):
    global _NC
    if _NC is None:
        _NC = build()
    return run_bass_kernel_spmd(_NC, _in_maps(inputs),
                                core_ids=list(range(N_CORES)), **kwargs)


def kernel(**inputs) -> np.ndarray:
    res = run_sharded(inputs)
    out = np.empty((B, NQ_FULL, CQ), np.float32)
    for core in range(N_CORES):
        b, half = core // 2, core % 2
        out[b, half * NQ:(half + 1) * NQ, :] = res.results[core]["out"]
    return out



# revision 18
# speedup vs baseline: 2.9279x; 1.6913x over previous
"""Trainium2 Bass kernel for CrossAttention (LN -> QKV proj -> MHA -> out proj).

Sharding: data-parallel over (batch, query-half): 8 shards for B=4.
Each core gets a [1024, 1024] query-token slice and the full [2048, 768]
context for its batch, and produces a [1024, 1024] output slice.

Per-core dataflow (matmul operands bf16, accumulation fp32 in PSUM):
  - LayerNorm on query/context tokens in natural [tok, C] layout (DVE
    bn_stats / bn_aggr), gamma/beta applied with partition-broadcast rows.
  - Transpose LN'd activations to channel-major [C, tok] via DMA xbar
    transpose (bf16) so channels sit on the contraction (partition) axis.
    Activations/projections are chunked (512 tokens) so projections start
    while later chunks are still normalizing.
  - QT = Wq^T @ XqT, KT = Wk^T @ XcT (channel-major), V natural [tok, C].
  - Attention runs per head-quad: scoresT[k, q] = K_h @ Q_h^T (contraction
    D=64; even/odd heads at partitions 0-63/64-127 land on disjoint PE row
    groups and overlap), exp on ACT straight out of PSUM (scores are
    bounded, so no max subtraction).  attendedT accumulates per head pair
    into one [128, 512] psum via column tiling (head0 rows 0-63, head1
    rows 64-127, concurrent).  Softmax denominators come from ones-vector
    matmuls, 4 heads packed at output partitions 0/32/64/96 of one psum.
  - Normalize with DVE reciprocal + DRAM-bounce partition broadcast, then
    out = attendedT^T @ Wo + bo in natural layout, DMA out.
"""

import numpy as np

import concourse.bass as bass
import concourse.tile as tile
from concourse import mybir
from concourse.bass_utils import run_bass_kernel_spmd

F32 = mybir.dt.float32
BF16 = mybir.dt.bfloat16
AF = mybir.ActivationFunctionType
OP = mybir.AluOpType

B, NQ_FULL, NK, CQ, CK, H, D = 4, 2048, 2048, 1024, 768, 16, 64
NQ = 1024            # per-core query tokens
N_CORES = 8
EPS = 1e-5
SM_SCALE = 1.0 / np.sqrt(D)  # 0.125

KC_Q = CQ // 128     # 8  contraction chunks for CQ
KC_C = CK // 128     # 6  contraction chunks for CK
NQT = NQ // 128      # 8  query token tiles
NKT = NK // 128      # 16 context token tiles
QC = 512             # q processed in chunks of 512 (psum free-dim limit)
NQ2 = NQ // QC       # 2


def _split_excess_waits(nc, max_waits=1):
    """walrus in this container accepts at most one sync wait per
    instruction; Tile's kernel-tail drain carries several.  Hoist excess
    waits onto single-wait NOPs that precede the instruction on the same
    engine (absolute sem waits commute, so this is semantics-preserving)."""
    for fn in nc.m.functions:
        for blk in fn.blocks:
            out = []
            dirty = False
            for inst in list(blk.instructions):
                si = inst.sync_info
                if si is not None and len(si.on_wait) > max_waits:
                    waits = list(si.on_wait)
                    for k, w in enumerate(waits[:-max_waits]):
                        nop = mybir.InstNoOp(
                            name=f"wsplit-{inst.name}-{k}", ins=[], outs=[])
                        nop.engine = inst.engine
                        nop.sync_info = mybir.SyncInfo(on_wait=[w], on_update=[])
                        out.append(nop)
                    inst.sync_info = mybir.SyncInfo(
                        on_wait=waits[-max_waits:], on_update=list(si.on_update))
                    dirty = True
                out.append(inst)
            if dirty:
                blk.instructions = out


def _bcast_ap(handle, n_parts, n_free):
    """DRAM [n_free] vector replicated across n_parts partitions."""
    return bass.AP(tensor=handle.ap().tensor, offset=0,
                   ap=[[0, n_parts], [1, n_free]])


def _emit(tc, t, out, stages=("proj", "attn", "out"), variant=()):
    from contextlib import ExitStack
    nc = tc.nc

    es = ExitStack()
    persist = es.enter_context(tc.tile_pool(name="persist", bufs=1))

    # chunked persistent tensors (distinct tags => distinct slots)
    # kT split per (channel-chunk, token-chunk): finer deps let attention
    # start while later context chunks are still projecting
    qTc = [persist.tile([128, NQ], BF16, tag=f"qT{oc}", name=f"qT{oc}")
           for oc in range(KC_Q)]
    kTc = [[persist.tile([128, QC], BF16, tag=f"kT{oc}_{t4}",
                         name=f"kT{oc}_{t4}") for t4 in range(NK // QC)]
           for oc in range(KC_Q)]
    # V with a ones column appended per head: the attended-value matmul
    # then also emits the softmax denominator (row 64 of its psum)
    v_g = [persist.tile([128, 4, H, D + 1], BF16, tag=f"v{g}", name=f"v{g}")
           for g in range(NKT // 4)]
    bq_cols = persist.tile([128, KC_Q], F32)
    bk_cols = persist.tile([128, KC_Q], F32)
    bvb = persist.tile([128, CQ], F32)
    eps_t = persist.tile([128, 1], F32)

    nc.vector.memset(eps_t[:, :], EPS)

    # wq + transposed query activations outlive the projection region: the
    # query projection is interleaved into the attention loop so ACT's exp
    # stream starts right after the context side is projected.
    wqp = es.enter_context(tc.tile_pool(name="wqp", bufs=1))
    xqTp = es.enter_context(tc.tile_pool(name="xqTp", bufs=2))

    # ---------------- phase 1+2: LN, transpose, K/V projections ------------
    with tc.tile_pool(name="pps", bufs=3, space="PSUM") as pps, \
         tc.tile_pool(name="lnw", bufs=1) as lnw, \
         tc.tile_pool(name="xfp", bufs=3) as xfp, \
         tc.tile_pool(name="stp", bufs=4) as stp, \
         tc.tile_pool(name="bfp", bufs=4) as bfp, \
         tc.tile_pool(name="wfp", bufs=3) as wfp, \
         tc.tile_pool(name="xTp", bufs=2) as xTp, \
         tc.tile_pool(name="wbp", bufs=1) as wbp, \
         tc.tile_pool(name="scrb", bufs=1, space="DRAM") as scrb:

        # gamma/beta as [128, n_chunk] columns; LN fold:
        # LN(x)@W + b = ((x-mu)*rstd) @ (gamma.W) + (b + (beta/gamma)@(gamma.W))
        gq = lnw.tile([128, KC_Q], F32, name="gq")
        gc = lnw.tile([128, KC_C], F32, name="gc")
        btq = lnw.tile([128, KC_Q], F32, name="btq")
        btc = lnw.tile([128, KC_C], F32, name="btc")
        nc.gpsimd.dma_start(out=gq[:, :],
                            in_=t["gamma_q"].ap().rearrange("(j p) -> p j", p=128))
        nc.gpsimd.dma_start(out=gc[:, :],
                            in_=t["gamma_ctx"].ap().rearrange("(j p) -> p j", p=128))
        nc.gpsimd.dma_start(out=btq[:, :],
                            in_=t["beta_q"].ap().rearrange("(j p) -> p j", p=128))
        nc.gpsimd.dma_start(out=btc[:, :],
                            in_=t["beta_ctx"].ap().rearrange("(j p) -> p j", p=128))
        btq8 = lnw.tile([128, KC_Q], BF16, name="btq8")
        btc8 = lnw.tile([128, KC_C], BF16, name="btc8")
        rg = lnw.tile([128, KC_Q], F32, name="rg")
        nc.vector.reciprocal(out=rg[:, :KC_Q], in_=gq[:, :])
        nc.vector.tensor_mul(out=btq8[:, :], in0=btq[:, :], in1=rg[:, :KC_Q])
        nc.vector.reciprocal(out=rg[:, :KC_C], in_=gc[:, :])
        nc.vector.tensor_mul(out=btc8[:, :], in0=btc[:, :], in1=rg[:, :KC_C])

        def ln_tile(x_dram, i, C, n_sub, sub, xT3, col0):
            """LN one [128, C] token tile (gamma/beta folded into the
            weights; input DMA-cast to bf16 on load so DVE runs in its
            2x/4x packed modes), then one fused xbar transpose writes all
            C//128 channel chunks of xT3 (out[p, j, t] = x[t, j*128+p])."""
            xb = xfp.tile([128, C], BF16, tag="xf", name=f"xf_{i}_{C}")
            nc.gpsimd.dma_start(out=xb[:, :],
                                in_=x_dram.ap()[i * 128:(i + 1) * 128, :])
            xbf = bfp.tile([128, C], BF16, tag="xbf", name=f"xbf_{i}_{C}")
            if "fast_ln" in variant:  # timing probe only: skip LN math
                nc.vector.tensor_copy(out=xbf[:, :], in_=xb[:, :])
            else:
                st = stp.tile([128, n_sub, 6], F32, tag="st",
                              name=f"st_{i}_{C}")
                for s in range(n_sub):
                    nc.vector.bn_stats(out=st[:, s, :],
                                       in_=xb[:, s * sub:(s + 1) * sub])
                mv = stp.tile([128, 2], F32, tag="mv", name=f"mv_{i}_{C}")
                nc.vector.bn_aggr(out=mv[:, :], in_=st[:, :, :])
                nc.scalar.activation(out=mv[:, 1:2], in_=mv[:, 1:2],
                                     func=AF.Sqrt, bias=eps_t[:, :], scale=1.0)
                nc.vector.reciprocal(out=mv[:, 1:2], in_=mv[:, 1:2])
                nc.vector.tensor_scalar(out=xbf[:, :], in0=xb[:, :],
                                        scalar1=mv[:, 0:1], scalar2=mv[:, 1:2],
                                        op0=OP.subtract, op1=OP.mult)
            if "no_xpose" in variant:  # timing probe: plain copy, no xbar
                nc.sync.dma_start(
                    out=xT3[:, :, col0:col0 + 128],
                    in_=xbf[:, :].rearrange("p (j t) -> p j t", t=128))
                return
            nc.sync.dma_start(out=xT3[:, :, col0:col0 + 128],
                              in_=xbf[:, :], transpose=True)

        def load_w(dram, n_chunks, tagp, gcol, pool=None):
            tiles = []
            for kc in range(n_chunks):
                wf = wfp.tile([128, CQ], BF16, tag="wf", name=f"wf{tagp}{kc}")
                nc.gpsimd.dma_start(out=wf[:, :],
                                    in_=dram.ap()[kc * 128:(kc + 1) * 128, :])
                wb = (pool or wbp).tile([128, CQ], BF16, tag=f"w{tagp}{kc}",
                                        name=f"w{tagp}{kc}")
                nc.scalar.activation(out=wb[:, :], in_=wf[:, :],
                                     func=AF.Copy, scale=gcol[:, kc:kc + 1])
                tiles.append(wb)
            return tiles

        def bias_fold(tiles, bname, n_kc, btcol, tagp, scr_pool):
            """bias' = b + beta~ @ W' -> DRAM row [CQ]."""
            brow = lnw.tile([1, CQ], F32, tag="brow",
                            name=f"brow{tagp}")
            nc.gpsimd.dma_start(out=brow[:, :],
                                in_=bass.AP(tensor=t[bname].ap().tensor,
                                            offset=0, ap=[[0, 1], [1, CQ]]))
            bsum = lnw.tile([1, CQ], F32, tag="bsum",
                            name=f"bsum{tagp}")
            for half in range(2):
                ps = pps.tile([1, QC], F32, tag="pp", name=f"bps{tagp}{half}")
                for kc in range(n_kc):
                    nc.tensor.matmul(ps[:, :], btcol[:, kc:kc + 1],
                                     tiles[kc][:, half * QC:(half + 1) * QC],
                                     start=(kc == 0), stop=(kc == n_kc - 1))
                nc.vector.tensor_tensor(out=bsum[:, half * QC:(half + 1) * QC],
                                        in0=ps[:, :],
                                        in1=brow[:, half * QC:(half + 1) * QC],
                                        op=OP.add)
            bd = scr_pool.tile([1, CQ], F32, tag=f"bd{tagp}", name=f"bd{tagp}")
            nc.gpsimd.dma_start(out=bd[:, :], in_=bsum[:, :])
            return bd

        # transposed query activations (consumed by the interleaved query
        # projection inside the attention loop)
        xqTs = [xqTp.tile([128, KC_Q, QC], BF16, tag="xqT", name=f"xqT_{t2}")
                for t2 in range(NQ2)]
        xcTs = [xTp.tile([128, KC_C, QC], BF16, tag="xcT", name=f"xcT_{t4}")
                for t4 in range(NK // QC)]

        def ln_ctx(t4):
            for i in range(4):
                ln_tile(t["xc"], t4 * 4 + i, CK, 3, 256, xcTs[t4], i * 128)

        # first context chunk's loads lead the Pool queue so LN starts
        # before the weight prefetches
        ln_ctx(0)

        wk = load_w(t["Wk"], KC_C, "k", gc)
        wv = load_w(t["Wv"], KC_C, "v", gc)
        bdk = bias_fold(wk, "bk", KC_C, btc8, "k", scrb)
        bdkap = bdk[0:1, :]
        nc.gpsimd.dma_start(
            out=bk_cols[:, :],
            in_=bass.AP(tensor=bdkap.tensor, offset=bdkap.offset,
                        ap=[[1, 128], [128, KC_Q]]))
        bdv = bias_fold(wv, "bv", KC_C, btc8, "v", scrb)
        bdvap = bdv[0:1, :]
        nc.gpsimd.dma_start(
            out=bvb[:, :],
            in_=bass.AP(tensor=bdvap.tensor, offset=bdvap.offset,
                        ap=[[0, 128], [1, CQ]]))

        # context side, software-pipelined: LN/transpose of chunk t4+1 (and
        # two query-token tiles) is emitted before chunk t4's projections so
        # the in-order DVE queue works ahead while PE runs matmuls
        for t4 in range(NK // QC):
            xcT = xcTs[t4]
            if t4 + 1 < NK // QC:
                ln_ctx(t4 + 1)
            for i in range(2):
                qi = 2 * t4 + i
                ln_tile(t["xq"], qi, CQ, 2, 512, xqTs[qi // 4],
                        (qi % 4) * 128)
            for oc in range(KC_Q):
                ps = pps.tile([128, QC], F32, tag="pp", name=f"psk{oc}_{t4}")
                for kc in range(KC_C):
                    nc.tensor.matmul(ps[:, :],
                                     wk[kc][:, oc * 128:(oc + 1) * 128],
                                     xcT[:, kc, :],
                                     start=(kc == 0), stop=(kc == KC_C - 1))
                nc.vector.tensor_scalar_add(
                    out=kTc[oc][t4][:, :], in0=ps[:, :],
                    scalar1=bk_cols[:, oc:oc + 1])
            for ki in range(4):
                kt = t4 * 4 + ki
                for v2 in range(CQ // QC):
                    ps = pps.tile([128, QC], F32, tag="pp",
                                  name=f"psv{kt}_{v2}")
                    for kc in range(KC_C):
                        nc.tensor.matmul(ps[:, :],
                                         xcT[:, kc, ki * 128:(ki + 1) * 128],
                                         wv[kc][:, v2 * QC:(v2 + 1) * QC],
                                         start=(kc == 0), stop=(kc == KC_C - 1))
                    nc.vector.tensor_tensor(
                        out=v_g[t4][:, ki, v2 * 8:(v2 + 1) * 8, 0:D],
                        in0=ps[:, :].rearrange("p (h d) -> p h d", d=D),
                        in1=bvb[:, v2 * QC:(v2 + 1) * QC].rearrange(
                            "p (h d) -> p h d", d=D),
                        op=OP.add)
                nc.vector.memset(v_g[t4][:, ki, :, D:D + 1], 1.0)

        # query-side weights + folded bias (projection happens later,
        # interleaved with attention)
        wq = load_w(t["Wq"], KC_Q, "q", gq, pool=wqp)
        bdq = bias_fold(wq, "bq", KC_Q, btq8, "q", scrb)
        bdqap = bdq[0:1, :]
        nc.gpsimd.dma_start(
            out=bq_cols[:, :],
            in_=bass.AP(tensor=bdqap.tensor, offset=bdqap.offset,
                        ap=[[1, 128], [128, KC_Q]]))

    # ---------------- phase 3: attention ----------------
    if "attn" not in stages:
        # timing-only partial build: flush something derived to out
        with tc.tile_pool(name="fl", bufs=1) as fl:
            fb = fl.tile([128, QC], F32, name="fb")
            nc.vector.tensor_copy(out=fb[:, :], in_=kTc[0][0][:, 0:QC])
            nc.sync.dma_start(out=out.ap()[0:128, 0:QC], in_=fb[:, :])
        es.close()
        return
    late = es.enter_context(tc.tile_pool(name="late", bufs=1))
    attT = late.tile([128, KC_Q, NQ], BF16, name="attT")
    wo = late.tile([128, KC_Q, CQ], BF16, name="wo")
    bob = late.tile([128, CQ], F32, name="bob")

    with tc.tile_pool(name="scps", bufs=2, space="PSUM") as scps, \
         tc.tile_pool(name="attps", bufs=2, space="PSUM") as attps, \
         tc.tile_pool(name="ep", bufs=4) as ep, \
         tc.tile_pool(name="rp", bufs=4) as rp, \
         tc.tile_pool(name="tmp1", bufs=2) as tmp1p, \
         tc.tile_pool(name="scr", bufs=4, space="DRAM") as scr:

        nc.gpsimd.dma_start(out=bob[:, :], in_=_bcast_ap(t["bo"], 128, CQ))
        for kc in range(KC_Q):
            nc.gpsimd.dma_start(out=wo[:, kc, :],
                                in_=t["Wo"].ap()[kc * 128:(kc + 1) * 128, :])

        def qproj(oc):
            # query projection for one head-pair channel chunk, emitted
            # from inside the attention loop (PE has slack there; ACT is
            # the bottleneck).  Borrows a scores psum slot.
            ps = scps.tile([128, NQ], F32, tag="sc", name=f"psq{oc}")
            for t2 in range(NQ2):
                for kc in range(KC_Q):
                    nc.tensor.matmul(ps[:, t2 * QC:(t2 + 1) * QC],
                                     wq[kc][:, oc * 128:(oc + 1) * 128],
                                     xqTs[t2][:, kc, :],
                                     start=(kc == 0), stop=(kc == KC_Q - 1))
            nc.vector.tensor_scalar_add(out=qTc[oc][:, :], in0=ps[:, :],
                                        scalar1=bq_cols[:, oc:oc + 1])

        qproj(0)
        for hp in range(H // 2):
            att = {}
            for par in range(2):
                h = 2 * hp + par
                att[par] = attps.tile([D + 1, NQ], F32, tag="att",
                                      name=f"attp{h}")

            def att_mm(kt, es):
                # attended + softmax denominator in one matmul:
                # lhsT = [V_h | ones], row 64 of psum = sum(exp)
                for par in range(2):
                    for q2 in range(NQ2):
                        nc.tensor.matmul(
                            att[par][:, q2 * QC:(q2 + 1) * QC],
                            v_g[kt // 4][:, kt % 4, 2 * hp + par, :],
                            es[par][:, q2 * QC:(q2 + 1) * QC],
                            start=(kt == 0), stop=(kt == NKT - 1))

            # software-pipelined: emit scores+exp for kt before the
            # attended matmuls of kt-1, so the (in-order) PE queue never
            # blocks on ACT's exp — sc(kt+1) runs while exp(kt) is busy,
            # and ACT stays saturated (it is the bottleneck here).
            pend = None
            for kt in range(NKT):
                cur = []
                for par in range(2):
                    h, lo = 2 * hp + par, par * 64
                    sc = scps.tile([128, NQ], F32, tag="sc",
                                   name=f"sc{h}_{kt}")
                    for q2 in range(NQ2):
                        nc.tensor.matmul(
                            sc[:, q2 * QC:(q2 + 1) * QC],
                            kTc[hp][kt // 4][lo:lo + 64,
                                             (kt % 4) * 128:(kt % 4 + 1) * 128],
                            qTc[hp][lo:lo + 64, q2 * QC:(q2 + 1) * QC],
                            start=True, stop=True)
                    # one exp over the full q width (both psum banks):
                    # halves the per-instruction ACT overhead
                    e = ep.tile([128, NQ], BF16, tag="e", name=f"e{h}_{kt}")
                    nc.scalar.activation(out=e[:, :], in_=sc[:, :],
                                         func=AF.Exp, scale=SM_SCALE)
                    cur.append(e)
                if pend is not None:
                    att_mm(pend[0], pend[1])
                if kt == 5 and hp + 1 < H // 2:
                    qproj(hp + 1)
                pend = (kt, cur)
            att_mm(pend[0], pend[1])
            for par in range(2):
                h = 2 * hp + par
                # drain psum to SBUF right away so the accumulator slot
                # frees for the next head pair; the (slow) normalize chain
                # then runs off the SBUF copy, off the critical path
                # (bf16 throughout: attT is bf16 anyway, and it halves both
                # the SBUF footprint and the DVE cost)
                atc = rp.tile([64, NQ], BF16, tag="atc", name=f"atc{h}")
                nc.vector.tensor_copy(out=atc[:, :], in_=att[par][0:D, :])
                rec = rp.tile([65, NQ], F32, tag="rec", name=f"rec{h}")
                nc.vector.reciprocal(out=rec[64:65, :], in_=att[par][64:65, :])
                sd = scr.tile([1, NQ], BF16, tag="sd", name=f"sd{h}")
                nc.gpsimd.dma_start(out=sd[:, :], in_=rec[64:65, :])
                rb = rp.tile([64, NQ], BF16, tag="rb", name=f"rb{h}")
                nc.gpsimd.dma_start(
                    out=rb[:, :],
                    in_=bass.AP(tensor=sd.tensor, offset=sd.offset,
                                ap=[[0, 64], [1, NQ]]))
                if par == 0:
                    nc.vector.tensor_mul(out=attT[0:64, hp, :],
                                         in0=atc[:, :], in1=rb[:, :])
                else:
                    # odd head: normalize at partitions 0-63, then DMA
                    # shifts it to partitions 64-127 of the attT chunk
                    tm = tmp1p.tile([64, NQ], BF16, tag="tm", name=f"tm{h}")
                    nc.vector.tensor_mul(out=tm[:, :],
                                         in0=atc[:, :], in1=rb[:, :])
                    nc.sync.dma_start(out=attT[64:128, hp, :], in_=tm[:, :])

    # ---------------- phase 4: out projection ----------------
    with tc.tile_pool(name="ops", bufs=2, space="PSUM") as ops, \
         tc.tile_pool(name="op", bufs=2) as op_pool:
        if "out" not in stages:
            fb2 = op_pool.tile([128, QC], F32, name="fb2")
            nc.vector.tensor_copy(out=fb2[:, :], in_=attT[:, 0, 0:QC])
            nc.sync.dma_start(out=out.ap()[0:128, 0:QC], in_=fb2[:, :])
        for qt in range(NQT if "out" in stages else 0):
            osb = op_pool.tile([128, CQ], F32, tag="osb", name=f"osb{qt}")
            for cc in range(CQ // QC):
                ps = ops.tile([128, QC], F32, tag="opp", name=f"pso{qt}_{cc}")
                for kc in range(KC_Q):
                    nc.tensor.matmul(
                        ps[:, :],
                        attT[:, kc, qt * 128:(qt + 1) * 128],
                        wo[:, kc, cc * QC:(cc + 1) * QC],
                        start=(kc == 0), stop=(kc == KC_Q - 1))
                nc.vector.tensor_tensor(out=osb[:, cc * QC:(cc + 1) * QC],
                                        in0=ps[:, :],
                                        in1=bob[:, cc * QC:(cc + 1) * QC],
                                        op=OP.add)
            nc.sync.dma_start(out=out.ap()[qt * 128:(qt + 1) * 128, :],
                              in_=osb[:, :])

    es.close()


def build(split_waits=True):
    nc = bass.Bass("TRN2", target_bir_lowering=False, debug=False,
                   num_devices=N_CORES)
    t = {
        "xq": nc.dram_tensor("xq", [NQ, CQ], F32, kind="ExternalInput"),
        "xc": nc.dram_tensor("xc", [NK, CK], F32, kind="ExternalInput"),
        "Wq": nc.dram_tensor("Wq", [CQ, CQ], F32, kind="ExternalInput"),
        "Wk": nc.dram_tensor("Wk", [CK, CQ], F32, kind="ExternalInput"),
        "Wv": nc.dram_tensor("Wv", [CK, CQ], F32, kind="ExternalInput"),
        "Wo": nc.dram_tensor("Wo", [CQ, CQ], F32, kind="ExternalInput"),
        "bq": nc.dram_tensor("bq", [CQ], F32, kind="ExternalInput"),
        "bk": nc.dram_tensor("bk", [CQ], F32, kind="ExternalInput"),
        "bv": nc.dram_tensor("bv", [CQ], F32, kind="ExternalInput"),
        "bo": nc.dram_tensor("bo", [CQ], F32, kind="ExternalInput"),
        "gamma_q": nc.dram_tensor("gamma_q", [CQ], F32, kind="ExternalInput"),
        "beta_q": nc.dram_tensor("beta_q", [CQ], F32, kind="ExternalInput"),
        "gamma_ctx": nc.dram_tensor("gamma_ctx", [CK], F32, kind="ExternalInput"),
        "beta_ctx": nc.dram_tensor("beta_ctx", [CK], F32, kind="ExternalInput"),
    }
    out = nc.dram_tensor("out", [NQ, CQ], F32, kind="ExternalOutput")
    with tile.TileContext(nc) as tc:
        _emit(tc, t, out)
    if split_waits:
        _split_excess_waits(nc)
    return nc


_NC = None


def _in_maps(inputs):
    q = np.ascontiguousarray(np.asarray(inputs["query_tokens"], dtype=np.float32))
    c = np.ascontiguousarray(np.asarray(inputs["context_tokens"], dtype=np.float32))
    shared = {k: np.ascontiguousarray(np.asarray(inputs[k], dtype=np.float32))
              for k in ("Wq", "Wk", "Wv", "Wo", "bq", "bk", "bv", "bo",
                        "gamma_q", "beta_q", "gamma_ctx", "beta_ctx")}
    maps = []
    for core in range(N_CORES):
        b, half = core // 2, core % 2
        m = dict(shared)
        m["xq"] = np.ascontiguousarray(q[b, half * NQ:(half + 1) * NQ, :])
        m["xc"] = np.ascontiguousarray(c[b])
        maps.append(m)
    return maps


def run_sharded(inputs, **kwargs):
    global _NC
    if _NC is None:
        _NC = build()
    return run_bass_kernel_spmd(_NC, _in_maps(inputs),
                                core_ids=list(range(N_CORES)), **kwargs)


def kernel(**inputs) -> np.ndarray:
    res = run_sharded(inputs)
    out = np.empty((B, NQ_FULL, CQ), np.float32)
    for core in range(N_CORES):
        b, half = core // 2, core % 2
        out[b, half * NQ:(half + 1) * NQ, :] = res.results[core]["out"]
    return out

